# revision 31
# baseline (speedup 1.0000x reference)
"""MeshConv TRN2 kernel: 4-queue non-transpose SWDGE gathers in 2-tap phases.

Phase p covers edge-tile p//2; even phases gather taps (0,2) on queues (0,1),
odd phases taps (1,3) on queues (2,3). Adjacent phases use disjoint queue
pairs so up to 4 gathers generate concurrently, while DMASW lane reuse
(8 global lanes, 2 per phase) stretches to 4 phases of slack -- the
"waiter-passed" witness semaphores that serialized the 4-gather-per-tile
variant resolve long before the reusing gather dispatches.
"""

import os
import sys

sys.path.insert(0, "/opt/trn_rl_repo")

from contextlib import ExitStack

import ml_dtypes
import numpy as np

import concourse.bacc as bacc
import concourse.bass as bass
import concourse.tile as tile
from concourse import mybir

BF16 = ml_dtypes.bfloat16

P = 128
B, C, E, KT = 4, 128, 30000, 5
CO = 256
NCORES = 8
EH = E // 2
TILE = 2560
TSZ = (2560, 2560, 2560, 2560, 2560, 1536, 768)
NT = len(TSZ)
EPAD = sum(TSZ)
CH = 480
SZW = [sz // 16 for sz in TSZ]
IDXTOT = 4 * sum(SZW)

_LAST_RESULTS = None
_PROGRAM = None


def build_program(nt: int = NT) -> bass.Bass:
    nc = bacc.Bacc("TRN2", num_swdge_queues=4)
    xt = nc.declare_dram_parameter("xt", [E, C], mybir.dt.bfloat16, isOutput=False)
    x0 = nc.declare_dram_parameter("x0", [C, EPAD], mybir.dt.bfloat16, isOutput=False)
    idx = nc.declare_dram_parameter("idx", [P, IDXTOT], mybir.dt.int16, isOutput=False)
    wt = nc.declare_dram_parameter("wt", [P, KT * CO], mybir.dt.bfloat16, isOutput=False)
    bias = nc.declare_dram_parameter("bias", [P, 2], mybir.dt.float32, isOutput=False)
    out = nc.declare_dram_parameter("out", [CO, EH], mybir.dt.bfloat16, isOutput=True)

    with tile.TileContext(nc) as tc, ExitStack() as ctx:
        consts = ctx.enter_context(tc.tile_pool(name="consts", bufs=1))
        xpool = ctx.enter_context(tc.tile_pool(name="x0p", bufs=2))
        gpool = ctx.enter_context(tc.tile_pool(name="gath", bufs=3))
        tpool = ctx.enter_context(tc.tile_pool(name="texp", bufs=2))
        cpool = ctx.enter_context(tc.tile_pool(name="comb", bufs=2))
        opool = ctx.enter_context(tc.tile_pool(name="outs", bufs=2))
        dpool = ctx.enter_context(tc.tile_pool(name="dmy", bufs=2))
        psum = ctx.enter_context(tc.tile_pool(name="psum", bufs=3, space="PSUM"))

        idx0_t = consts.tile([P, 4 * SZW[0]], mybir.dt.int16, tag="idx0")
        nc.sync.dma_start(out=idx0_t[:], in_=idx[:, : 4 * SZW[0]])
        idxr_t = consts.tile([P, IDXTOT - 4 * SZW[0]], mybir.dt.int16, tag="idxr")
        nc.sync.dma_start(out=idxr_t[:], in_=idx[:, 4 * SZW[0] :])
        wt_t = consts.tile([P, KT * CO], mybir.dt.bfloat16)
        nc.scalar.dma_start(out=wt_t[:], in_=wt[:])
        bias_t = consts.tile([P, 2], mybir.dt.float32)
        nc.scalar.dma_start(out=bias_t[:], in_=bias[:])

        toff = [sum(TSZ[:i]) for i in range(NT + 1)]
        ioff = [4 * sum(SZW[:i]) for i in range(NT + 1)]

        nidx_reg = {}
        for s_ in sorted(set(TSZ[:nt])):
            r_ = ctx.enter_context(nc.gpsimd.register(name=f"nidx{s_}"))
            nc.gpsimd.reg_mov(r_, s_)
            nidx_reg[s_] = r_

        def emit_phase(t):
            sz = TSZ[t]
            szw = sz // 16
            idx_t = idx0_t if t == 0 else idxr_t
            ib = 0 if t == 0 else ioff[t] - 4 * SZW[0]
            g = {}
            gnames = bass.InstructionNameOrderedSet()
            for k in range(4):
                gk = gpool.tile([P, TILE], mybir.dt.bfloat16, tag=f"g{k}")
                gi = nc.gpsimd.dma_gather(
                    gk[:, :sz].rearrange("p (r c) -> p r c", c=C),
                    xt[:],
                    idx_t[:, ib + k * szw : ib + (k + 1) * szw],
                    num_idxs=sz,
                    num_idxs_reg=nidx_reg[sz],
                    elem_size=C,
                    transpose=False,
                    single_packet=False,
                    queue_num=k,
                )
                gnames.add(gi.ins.name)
                g[k] = gk
            mk = dpool.tile([P, 8], mybir.dt.float32, tag="mark")
            mark = nc.gpsimd.memset(mk[:, :1], 0.0)
            gnames.add(mark.ins.name)
            return g, gnames

        def emit_combines(t, g, gate_names):
            # Elementwise combines run directly in the gathered edge-partition
            # layout -- so the DVE ops (instant-pass, fast-witness) are the
            # only DMASW lane waiters, replicating the microbench structure
            # that sustained 4-wide gather generation.
            sz = TSZ[t]
            # No scheduling gate here: the combines are the DMASW lane
            # waiters, and delaying them delays the witness semaphores that
            # release the reusing gathers.
            pt = cpool.tile([P, TILE], mybir.dt.bfloat16, tag="p")
            nc.vector.tensor_tensor(
                out=pt[:, :sz], in0=g[0][:, :sz], in1=g[2][:, :sz], op=mybir.AluOpType.add
            )
            d13 = cpool.tile([P, TILE], mybir.dt.bfloat16, tag="d13")
            nc.vector.tensor_tensor(
                out=d13[:, :sz], in0=g[0][:, :sz], in1=g[2][:, :sz], op=mybir.AluOpType.subtract
            )
            qt = cpool.tile([P, TILE], mybir.dt.bfloat16, tag="q")
            nc.vector.tensor_tensor(
                out=qt[:, :sz], in0=g[1][:, :sz], in1=g[3][:, :sz], op=mybir.AluOpType.add
            )
            d24 = cpool.tile([P, TILE], mybir.dt.bfloat16, tag="d24")
            nc.vector.tensor_tensor(
                out=d24[:, :sz], in0=g[1][:, :sz], in1=g[3][:, :sz], op=mybir.AluOpType.subtract
            )
            nc.scalar.activation(
                out=d13[:, :sz], in_=d13[:, :sz], func=mybir.ActivationFunctionType.Abs
            )
            nc.scalar.activation(
                out=d24[:, :sz], in_=d24[:, :sz], func=mybir.ActivationFunctionType.Abs
            )
            return {1: pt, 2: qt, 3: d13, 4: d24}

        def emit_compute(t, cb, x0_t):
            sz = TSZ[t]
            tt = {}
            for k in (1, 2, 3, 4):
                tk = tpool.tile([P, TILE], mybir.dt.bfloat16, tag=f"t{k}")
                nc.sync.dma_start_transpose(
                    out=tk[:, :sz].rearrange("p (r c) -> p r c", c=C),
                    in_=cb[k][:, :sz],
                )
                tt[k] = tk
            wit = dpool.tile([P, 8], mybir.dt.float32, tag="wit")
            nc.sync.dma_start(out=wit[:, :1], in_=bias[:, :1])
            taps5 = [(0, x0_t), (1, tt[1]), (2, tt[2]), (3, tt[3]), (4, tt[4])]
            ob = [
                opool.tile([P, TILE], mybir.dt.bfloat16, tag=f"o{h}", name=f"ob{h}")
                for h in range(2)
            ]
            nch = (sz + CH - 1) // CH
            for ci in range(nch):
                w = min(CH, sz - ci * CH)
                for h in range(2):
                    ps = psum.tile([P, CH], mybir.dt.float32, tag=f"ps{h}")
                    for j, (k, rt) in enumerate(taps5):
                        nc.tensor.matmul(
                            out=ps[:, :w],
                            lhsT=wt_t[:, k * CO + h * P : k * CO + h * P + P],
                            rhs=rt[:, ci * CH : ci * CH + w],
                            start=(j == 0),
                            stop=(j == len(taps5) - 1),
                        )
                    nc.scalar.activation(
                        out=ob[h][:, ci * CH : ci * CH + w],
                        in_=ps[:, :w],
                        func=mybir.ActivationFunctionType.Identity,
                        bias=bias_t[:, h : h + 1],
                    )
            ncols = min(sz, EH - toff[t])
            for h in range(2):
                nc.scalar.dma_start(
                    out=out[h * P : (h + 1) * P, toff[t] : toff[t] + ncols],
                    in_=ob[h][:, :ncols],
                )

        # Pipeline: emit tile t's gathers, then tile t-1's combines (gated
        # after tile t's gathers in the schedule) + transposes + matmuls.
        pend = None          # (t, g, gnames, x0_t)
        for pi in range(nt + 1):
            if pi < nt:
                x0_t = xpool.tile([P, TILE], mybir.dt.bfloat16, tag="x0")
                nc.scalar.dma_start(
                    out=x0_t[:, : TSZ[pi]], in_=x0[:, toff[pi] : toff[pi] + TSZ[pi]]
                )
                g, gn = emit_phase(pi)
                cur = (pi, g, gn, x0_t)
            else:
                cur = None
            if pend is not None:
                pt_, g_, gn_, x0_ = pend
                gate = cur[2] if cur is not None else gn_
                cb = emit_combines(pt_, g_, gate)
                emit_compute(pt_, cb, x0_)
            pend = cur
    nc.finalize()
    return nc


def make_in_maps(x, ne_idx, conv_w, conv_b):
    xs = np.asarray(x)[..., 0]
    xtb = np.ascontiguousarray(xs.transpose(0, 2, 1)).astype(BF16)
    x0b = xs.astype(BF16)

    wt_host = np.zeros((P, KT * CO), np.float32)
    for k in range(KT):
        wt_host[:, k * CO : (k + 1) * CO] = conv_w[:, :, 0, k].T
    wt_host = wt_host.astype(BF16)
    bias_host = np.ascontiguousarray(np.asarray(conv_b).reshape(2, P).T).astype(
        np.float32
    )

    ne = np.asarray(ne_idx)
    in_maps = []
    for core in range(NCORES):
        b, h = divmod(core, 2)
        lo = h * EH
        x0c = np.zeros((C, EPAD), BF16)
        x0c[:, :EH] = x0b[b][:, lo : lo + EH]
        idxc = np.zeros((EH, 4), np.int16)
        idxc[:] = ne[b, lo : lo + EH, :].astype(np.int16)
        rep = np.zeros((P, IDXTOT), np.int16)
        woff = 0
        eoff = 0
        for t, sz in enumerate(TSZ):
            szw = sz // 16
            rsz = min(sz, EH - eoff)
            for k in range(4):
                ids = np.zeros(sz, np.int16)
                ids[:rsz] = idxc[eoff : eoff + rsz, k]
                blk = ids.reshape(szw, 16).T
                rep[:, woff : woff + szw] = np.tile(blk, (8, 1))
                woff += szw
            eoff += sz
        in_maps.append(
            {"xt": xtb[b], "x0": x0c, "idx": rep, "wt": wt_host, "bias": bias_host}
        )
    return in_maps


def kernel(x, ne_idx, conv_w, conv_b):
    global _LAST_RESULTS, _PROGRAM
    from concourse.bass_utils import run_bass_kernel_spmd

    in_maps = make_in_maps(x, ne_idx, conv_w, conv_b)
    if _PROGRAM is None:
        _PROGRAM = build_program()
    res = run_bass_kernel_spmd(
        _PROGRAM,
        in_maps,
        core_ids=list(range(NCORES)),
        trace=bool(os.environ.get("KERNEL_TRACE")),
    )
    _LAST_RESULTS = res

    out_full = np.zeros((B, CO, E), np.float32)
    for core in range(NCORES):
        b, h = divmod(core, 2)
        out_full[b, :, h * EH : (h + 1) * EH] = res.results[core]["out"].astype(
            np.float32
        )
    return out_full[..., None]
